# revision 11
# baseline (speedup 1.0000x reference)
"""Trainium2 Bass kernel for a dense transformer block (attention + MLP),
data-parallel over the batch dimension across 8 NeuronCores.

Reference semantics (per batch element, computed fully on one core):
    x  = rms_norm(latents) * ln1_scale
    q,k,v = x @ wq, x @ wk, x @ wv          (heads H=16, head_dim D=64)
    q  = rms_norm_d(q) * q_ln_scale / sqrt(D);  k = rms_norm_d(k) * k_ln_scale
    o  = softmax(q k^T) v ;  o = o @ wo ;  x2 = o + latents
    y  = rms_norm(x2) * ln2_scale
    out = gelu(y @ wi) @ wo_mlp + x2

Layout strategy: "T layout" ([feature, token]) for everything matmul-facing.
fp32 data runs through the PE as float32r (tf32-like, 1 cycle/row at N=512).
Softmax denominators ride as a ones-column appended to v, so attn@v yields
both o^T and the per-query sums in one accumulation group.  SBUF pressure is
managed with two LIFO pool stacks (left/right heap sides) so tensor lifetimes
can overlap across phases.  KPHASES=<n> truncates the kernel for bisection.
"""

import os

import numpy as np

import concourse.bass as bass
import concourse.mybir as mybir
import concourse.tile as tile
from concourse import bacc
from concourse.bass_utils import run_bass_kernel_spmd
from concourse.masks import make_identity

F32 = mybir.dt.float32
F32R = mybir.dt.float32r
BF16 = mybir.dt.bfloat16
AF = mybir.ActivationFunctionType
ALU = mybir.AluOpType

B, S, E, H, D, F = 8, 1024, 1024, 16, 64, 4096
HD = H * D            # 1024
ST = S // 128         # 8 token tiles
ET = E // 128         # 8 embedding tiles
FT = F // 128         # 32 mlp tiles
NCH = 512             # matmul moving-dim chunk (one psum bank of f32)
EPS = 1e-6


def r32(ap):
    return ap.bitcast(F32R)


def build():
    nphases = int(os.environ.get("KPHASES", "5"))
    nc = bacc.Bacc()

    lat_ext = nc.declare_dram_parameter("latents", [S, E], F32, isOutput=False)
    ln1_ext = nc.declare_dram_parameter("ln1_scale", [E], F32, isOutput=False)
    wq_ext = nc.declare_dram_parameter("wq", [E, HD], F32, isOutput=False)
    wk_ext = nc.declare_dram_parameter("wk", [E, HD], F32, isOutput=False)
    wv_ext = nc.declare_dram_parameter("wv", [E, HD], F32, isOutput=False)
    qls_ext = nc.declare_dram_parameter("q_ln_scale", [D], F32, isOutput=False)
    kls_ext = nc.declare_dram_parameter("k_ln_scale", [D], F32, isOutput=False)
    wo_ext = nc.declare_dram_parameter("wo", [HD, E], F32, isOutput=False)
    ln2_ext = nc.declare_dram_parameter("ln2_scale", [E], F32, isOutput=False)
    wi_ext = nc.declare_dram_parameter("wi", [E, F], F32, isOutput=False)
    wm_ext = nc.declare_dram_parameter("wo_mlp", [F, E], F32, isOutput=False)
    out_ext = nc.declare_dram_parameter("out", [S, E], F32, isOutput=True)

    def dbg_out(tc, src3, n, side):
        dbg = tc.alloc_tile_pool(name="dbg", bufs=1, side=side)
        for i in range(n):
            dt_ = dbg.tile([128, src3.shape[2]], F32, tag="dbgt", name=f"dbg{i}")
            nc.vector.tensor_copy(dt_[:], src3[:, i, :])
            nc.sync.dma_start(out_ext[i * 128:(i + 1) * 128, :], dt_[:])
        dbg.release()

    with tile.TileContext(nc) as tc:
        cst = tc.alloc_tile_pool(name="const", bufs=1, side="left")

        # ---- constants ----
        ident = cst.tile([128, 128], F32)
        make_identity(nc, ident[:])
        ones128_f = cst.tile([128, 1], F32)
        nc.vector.memset(ones128_f[:], 1.0)
        ones128 = cst.tile([128, 1], F32)          # f32r-rounded ones
        nc.scalar.copy(r32(ones128[:]), ones128_f[:])
        ln1s = cst.tile([128, ET], F32)            # ln1_scale, E on partitions
        nc.sync.dma_start(ln1s[:], ln1_ext[:].rearrange("(t p) -> p t", p=128))
        ln2s = cst.tile([128, ET], F32)
        nc.sync.dma_start(ln2s[:], ln2_ext[:].rearrange("(t p) -> p t", p=128))
        qls2 = cst.tile([128, 1], F32)             # q_ln_scale twice (2 heads/tile)
        nc.sync.dma_start(qls2[0:64, :], qls_ext[:].rearrange("(d o) -> d o", o=1))
        nc.sync.dma_start(qls2[64:128, :], qls_ext[:].rearrange("(d o) -> d o", o=1))
        kls2 = cst.tile([128, 1], F32)
        nc.sync.dma_start(kls2[0:64, :], kls_ext[:].rearrange("(d o) -> d o", o=1))
        nc.sync.dma_start(kls2[64:128, :], kls_ext[:].rearrange("(d o) -> d o", o=1))
        c_eps = cst.tile([128, 1], F32)
        nc.vector.memset(c_eps[:], EPS)
        c_eps64 = cst.tile([128, 1], F32)
        nc.vector.memset(c_eps64[:], D * EPS)

        # ================= Phase 1: x_norm + transpose =================
        xnT_p = tc.alloc_tile_pool(name="xnT_p", bufs=1, side="left")
        xnT = xnT_p.tile([128, ET, S], F32)        # x_norm^T [E, S], f32r-written

        p1 = tc.alloc_tile_pool(name="p1", bufs=1, side="left")
        p1b = tc.alloc_tile_pool(name="p1b", bufs=3, side="left")
        p1ps = tc.alloc_tile_pool(name="p1ps", bufs=2, space="PSUM")

        xn = p1.tile([128, ST, E], F32)
        rms = p1.tile([128, 3 * ST], F32)   # ssq | sqrt | recip
        for t in range(ST):
            latt = p1b.tile([128, E], F32, tag="latt", name=f"latt{t}")
            nc.sync.dma_start(latt[:], lat_ext[t * 128:(t + 1) * 128, :])
            scr = p1b.tile([128, E], F32, tag="scr1", name=f"scr1_{t}")
            nc.vector.tensor_mul(scr[:], latt[:], latt[:])
            nc.vector.reduce_sum(rms[:, t:t + 1], scr[:], axis=mybir.AxisListType.X)
            nc.scalar.activation(rms[:, ST + t:ST + t + 1], rms[:, t:t + 1],
                                 AF.Sqrt, bias=c_eps[:], scale=1.0 / E)
            nc.vector.reciprocal(rms[:, 2 * ST + t:2 * ST + t + 1],
                                 rms[:, ST + t:ST + t + 1])
            nc.vector.tensor_scalar_mul(xn[:, t, :], latt[:],
                                        rms[:, 2 * ST + t:2 * ST + t + 1])
        # transpose x_norm -> xnT, folding ln1_scale into the evac
        for e in range(ET):
            tp = p1ps.tile([128, S], F32, tag="tp1", name=f"tp1_{e}")
            for t in range(ST):
                nc.tensor.transpose(tp[:, t * 128:(t + 1) * 128],
                                    xn[:, t, e * 128:(e + 1) * 128], ident[:])
            nc.vector.tensor_scalar_mul(r32(xnT[:, e, :]), tp[:],
                                        ln1s[:, e:e + 1])
        p1ps.release()
        p1b.release()
        p1.release()

        if nphases == 1:
            dbg_out(tc, xnT, ET, "right")
            xnT_p.release()
            cst.release()

        # ================= Phase 2: QKV projections + q/k norms ========
        if nphases >= 2:
            qkv_p = tc.alloc_tile_pool(name="qkv_p", bufs=1, side="right")
            qT = qkv_p.tile([128, ET, S], F32)     # q^T [HD, S], f32r-written
            kT = qkv_p.tile([128, ET, S], F32)
            v_sb = qkv_p.tile([128, ST, H * (D + 1)], BF16)  # v + ones col

            p2w = tc.alloc_tile_pool(name="p2w", bufs=8, side="left")
            p2 = tc.alloc_tile_pool(name="p2", bufs=2, side="right")
            p2s = tc.alloc_tile_pool(name="p2s", bufs=1, side="right")
            p2ps = tc.alloc_tile_pool(name="p2ps", bufs=2, space="PSUM")
            p2ps1 = tc.alloc_tile_pool(name="p2ps1", bufs=1, space="PSUM")

            for name, w_ext_, outT, scl2 in (
                ("q", wq_ext, qT, qls2), ("k", wk_ext, kT, kls2),
            ):
                wsl = [p2w.tile([128, HD], F32, tag="wsl", name=f"wsl{name}{i}")
                       for i in range(ET)]
                for kk in range(ET):
                    nc.sync.dma_start(r32(wsl[kk][:]),
                                      r32(w_ext_[kk * 128:(kk + 1) * 128, :]))
                for m in range(ET):
                    ps = p2ps.tile([128, S], F32, tag="proj", name=f"proj{name}{m}")
                    for kk in range(ET):
                        for c in range(2):
                            ch = slice(c * NCH, (c + 1) * NCH)
                            nc.tensor.matmul(
                                ps[:, ch],
                                r32(wsl[kk][:, m * 128:(m + 1) * 128]),
                                r32(xnT[:, kk, ch]),
                                start=(kk == 0), stop=(kk == ET - 1),
                            )
                    # sum of squares over d (64-part blocks) via ones-matmul
                    scr = p2.tile([128, S], F32, tag="scr2", name=f"scr2{name}{m}")
                    nc.scalar.activation(r32(scr[:]), ps[:], AF.Square)
                    ssqA = p2ps1.tile([1, S], F32, tag="ssqA", name=f"ssqA{name}{m}")
                    ssqB = p2ps1.tile([1, S], F32, tag="ssqB", name=f"ssqB{name}{m}")
                    for c in range(2):
                        ch = slice(c * NCH, (c + 1) * NCH)
                        nc.tensor.matmul(ssqA[:, ch], r32(ones128[0:64, :]),
                                         r32(scr[0:64, ch]), start=True, stop=True)
                        nc.tensor.matmul(ssqB[:, ch], r32(ones128[64:128, :]),
                                         r32(scr[64:128, ch]), start=True, stop=True)
                    # q: 1/(8*sqrt(mean+eps)) = recip(sqrt(ssq*1 + 64eps))
                    # k: 1/sqrt(mean+eps)     = recip(sqrt(ssq/64 + eps))
                    sc = 1.0 if name == "q" else 1.0 / D
                    bi = c_eps64[0:1, :] if name == "q" else c_eps[0:1, :]
                    sqA = p2s.tile([1, S], F32, tag="sqA", name=f"sqA{name}{m}")
                    sqB = p2s.tile([1, S], F32, tag="sqB", name=f"sqB{name}{m}")
                    nc.scalar.activation(sqA[:], ssqA[:], AF.Sqrt, bias=bi, scale=sc)
                    nc.scalar.activation(sqB[:], ssqB[:], AF.Sqrt, bias=bi, scale=sc)
                    rA = p2s.tile([1, S], F32, tag="rA", name=f"rA{name}{m}")
                    rB = p2s.tile([1, S], F32, tag="rB", name=f"rB{name}{m}")
                    nc.vector.reciprocal(rA[:], sqA[:])
                    nc.vector.reciprocal(rB[:], sqB[:])
                    bcA = p2.tile([64, S], F32, tag="bcA", name=f"bcA{name}{m}")
                    bcB = p2.tile([64, S], F32, tag="bcB", name=f"bcB{name}{m}")
                    nc.gpsimd.partition_broadcast(bcA[:], rA[:])
                    nc.gpsimd.partition_broadcast(bcB[:], rB[:])
                    # scl2's halves hold identical values; base-0 slices keep
                    # all SB inputs on one partition window (NCC_IBIR297).
                    nc.vector.scalar_tensor_tensor(
                        out=r32(outT[0:64, m, :]), in0=ps[0:64, :],
                        scalar=scl2[0:64, :], in1=bcA[:],
                        op0=ALU.mult, op1=ALU.mult)
                    nc.vector.scalar_tensor_tensor(
                        out=r32(outT[64:128, m, :]), in0=ps[64:128, :],
                        scalar=scl2[0:64, :], in1=bcB[:],
                        op0=ALU.mult, op1=ALU.mult)

            # v in [S, HD] layout: lhsT = xnT tiles (stationary), rhs = wv slab
            wsl = [p2w.tile([128, HD], F32, tag="wsl", name=f"wslv{i}")
                   for i in range(ET)]
            for kk in range(ET):
                nc.sync.dma_start(r32(wsl[kk][:]),
                                  r32(wv_ext[kk * 128:(kk + 1) * 128, :]))
            v3 = v_sb[:].rearrange("p t (h c) -> p t h c", c=D + 1)
            for m in range(ST):
                ps = p2ps.tile([128, S], F32, tag="proj", name=f"projv{m}")
                for kk in range(ET):
                    for c in range(2):
                        ch = slice(c * NCH, (c + 1) * NCH)
                        nc.tensor.matmul(
                            ps[:, ch],
                            r32(xnT[:, kk, m * 128:(m + 1) * 128]),
                            r32(wsl[kk][:, ch]),
                            start=(kk == 0), stop=(kk == ET - 1),
                        )
                nc.vector.tensor_copy(
                    v3[:, m, :, 0:D],
                    ps[:].rearrange("p (h c) -> p h c", c=D))
                nc.vector.memset(v3[:, m, :, D:D + 1], 1.0)

            p2ps1.release()
            p2ps.release()
            p2s.release()
            p2.release()
            p2w.release()
            xnT_p.release()

            if nphases == 2:
                dbg_out(tc, qT, ET, "left")
                qkv_p.release()
                cst.release()

        # ================= Phase 3: attention ==========================
        if nphases >= 3:
            oT_p = tc.alloc_tile_pool(name="oT_p", bufs=1, side="left")
            oT = oT_p.tile([128, ET, S], F32)      # o^T [HD, S], f32r-written

            p3 = tc.alloc_tile_pool(name="p3", bufs=2, side="right")
            p3m = tc.alloc_tile_pool(name="p3m", bufs=2, side="left")
            p3ps = tc.alloc_tile_pool(name="p3ps", bufs=1, space="PSUM")
            p3po = tc.alloc_tile_pool(name="p3po", bufs=2, space="PSUM")

            v3 = v_sb[:].rearrange("p t (h c) -> p t h c", c=D + 1)
            for hp in range(H // 2):      # head pairs (2*hp, 2*hp+1)
                expA = p3.tile([128, ST, S], BF16, tag="expA", name=f"expA{hp}")
                expB = p3.tile([128, ST, S], BF16, tag="expB", name=f"expB{hp}")
                for skt in range(ST):
                    sks = slice(skt * 128, (skt + 1) * 128)
                    lgA = p3ps.tile([128, S], F32, tag="lgA", name=f"lgA{hp}_{skt}")
                    lgB = p3ps.tile([128, S], F32, tag="lgB", name=f"lgB{hp}_{skt}")
                    for c in range(2):
                        ch = slice(c * NCH, (c + 1) * NCH)
                        nc.tensor.matmul(lgA[:, ch], r32(kT[0:64, hp, sks]),
                                         r32(qT[0:64, hp, ch]),
                                         start=True, stop=True)
                        nc.tensor.matmul(lgB[:, ch], r32(kT[64:128, hp, sks]),
                                         r32(qT[64:128, hp, ch]),
                                         start=True, stop=True)
                    nc.scalar.activation(expA[:, skt, :], lgA[:], AF.Exp)
                    nc.scalar.activation(expB[:, skt, :], lgB[:], AF.Exp)
                for half, expX in ((0, expA), (1, expB)):
                    h = 2 * hp + half
                    oa = p3po.tile([128, S], F32, tag="oacc", name=f"oacc{h}")
                    for skt in range(ST):
                        for c in range(2):
                            ch = slice(c * NCH, (c + 1) * NCH)
                            nc.tensor.matmul(
                                oa[0:D + 1, ch], v3[:, skt, h, :],
                                expX[:, skt, ch],
                                start=(skt == 0), stop=(skt == ST - 1))
                    sums = p3m.tile([1, S], F32, tag="sums", name=f"sums{h}")
                    nc.scalar.copy(sums[:], oa[D:D + 1, :])
                    rs = p3m.tile([1, S], F32, tag="rs", name=f"rs{h}")
                    nc.vector.reciprocal(rs[:], sums[:])
                    bco = p3m.tile([64, S], F32, tag="bco", name=f"bco{h}")
                    nc.gpsimd.partition_broadcast(bco[:], rs[:])
                    nc.vector.tensor_tensor(
                        r32(oT[half * 64:(half + 1) * 64, hp, :]),
                        oa[0:D, :], bco[:], ALU.mult)

            p3po.release()
            p3ps.release()
            p3m.release()
            p3.release()
            qkv_p.release()

            if nphases == 3:
                dbg_out(tc, oT, ET, "right")
                oT_p.release()
                cst.release()

        # ============ Phase 4: o-proj + residual + ln2 + transpose =====
        if nphases >= 4:
            x2_p = tc.alloc_tile_pool(name="x2_p", bufs=1, side="right")
            x2 = x2_p.tile([128, ST, E], F32)      # attn residual [S, E]
            yT_p = tc.alloc_tile_pool(name="yT_p", bufs=1, side="right")
            yT = yT_p.tile([128, ET, S], F32)      # rms(x2)^T, f32r-written

            p4w = tc.alloc_tile_pool(name="p4w", bufs=8, side="left")
            p4 = tc.alloc_tile_pool(name="p4", bufs=1, side="left")
            p4b = tc.alloc_tile_pool(name="p4b", bufs=2, side="left")
            p4ps = tc.alloc_tile_pool(name="p4ps", bufs=2, space="PSUM")

            wsl = [p4w.tile([128, E], F32, tag="wosl", name=f"wosl{i}")
                   for i in range(ET)]
            for kk in range(ET):
                nc.sync.dma_start(r32(wsl[kk][:]),
                                  r32(wo_ext[kk * 128:(kk + 1) * 128, :]))
            y = p4.tile([128, ST, E], F32)
            rms2 = p4.tile([128, 3 * ST], F32)
            for m in range(ST):
                lat2 = p4b.tile([128, E], F32, tag="lat2", name=f"lat2_{m}")
                nc.sync.dma_start(lat2[:], lat_ext[m * 128:(m + 1) * 128, :])
                ps = p4ps.tile([128, E], F32, tag="oproj", name=f"oproj{m}")
                for kk in range(ET):
                    for c in range(2):
                        ch = slice(c * NCH, (c + 1) * NCH)
                        nc.tensor.matmul(
                            ps[:, ch],
                            r32(oT[:, kk, m * 128:(m + 1) * 128]),
                            r32(wsl[kk][:, ch]),
                            start=(kk == 0), stop=(kk == ET - 1),
                        )
                nc.vector.tensor_add(x2[:, m, :], ps[:], lat2[:])
                scr = p4b.tile([128, E], F32, tag="scr4", name=f"scr4_{m}")
                nc.vector.tensor_mul(scr[:], x2[:, m, :], x2[:, m, :])
                nc.vector.reduce_sum(rms2[:, m:m + 1], scr[:],
                                     axis=mybir.AxisListType.X)
                nc.scalar.activation(rms2[:, ST + m:ST + m + 1], rms2[:, m:m + 1],
                                     AF.Sqrt, bias=c_eps[:], scale=1.0 / E)
                nc.vector.reciprocal(rms2[:, 2 * ST + m:2 * ST + m + 1],
                                     rms2[:, ST + m:ST + m + 1])
                nc.vector.tensor_scalar_mul(y[:, m, :], x2[:, m, :],
                                            rms2[:, 2 * ST + m:2 * ST + m + 1])
            p4ps.release()
            p4tps = tc.alloc_tile_pool(name="p4tps", bufs=2, space="PSUM")
            for e in range(ET):
                tp = p4tps.tile([128, S], F32, tag="tp4", name=f"tp4_{e}")
                for t in range(ST):
                    nc.tensor.transpose(tp[:, t * 128:(t + 1) * 128],
                                        y[:, t, e * 128:(e + 1) * 128], ident[:])
                nc.vector.tensor_scalar_mul(r32(yT[:, e, :]), tp[:],
                                            ln2s[:, e:e + 1])
            p4tps.release()
            p4b.release()
            p4.release()
            p4w.release()
            oT_p.release()

            if nphases == 4:
                dbg_out(tc, x2, ST, "left")
                yT_p.release()
                x2_p.release()
                cst.release()

        # ================= Phase 5: MLP ================================
        if nphases >= 5:
            h1_p = tc.alloc_tile_pool(name="h1_p", bufs=1, side="right")
            h1 = h1_p.tile([128, FT, S], BF16)     # gelu(fc1) [F, S]
            wmb_p = tc.alloc_tile_pool(name="wmb_p", bufs=1, side="right")
            wmb = wmb_p.tile([128, FT, E], BF16)   # wo_mlp, bf16

            p5w = tc.alloc_tile_pool(name="p5w", bufs=8, side="left")
            p5ps = tc.alloc_tile_pool(name="p5ps", bufs=2, space="PSUM")
            for mf in range(FT):
                ps = p5ps.tile([128, S], F32, tag="fc1", name=f"fc1_{mf}")
                for kk in range(ET):
                    wt = p5w.tile([128, 128], F32, tag="wisl",
                                  name=f"wisl{mf}_{kk}")
                    nc.sync.dma_start(
                        r32(wt[:]),
                        r32(wi_ext[kk * 128:(kk + 1) * 128,
                                   mf * 128:(mf + 1) * 128]))
                    for c in range(2):
                        ch = slice(c * NCH, (c + 1) * NCH)
                        nc.tensor.matmul(ps[:, ch], r32(wt[:]),
                                         r32(yT[:, kk, ch]),
                                         start=(kk == 0), stop=(kk == ET - 1))
                nc.scalar.activation(h1[:, mf, :], ps[:], AF.Gelu_apprx_tanh)
            p5ps.release()
            p5w.release()

            # fc2 (bf16) + final residual
            p5m = tc.alloc_tile_pool(name="p5m", bufs=2, side="left")
            p5o = tc.alloc_tile_pool(name="p5o", bufs=1, side="left")
            p5po = tc.alloc_tile_pool(name="p5po", bufs=2, space="PSUM")
            for kk in range(FT):
                wf = p5m.tile([128, E], F32, tag="wmf", name=f"wmf{kk}")
                nc.sync.dma_start(wf[:], wm_ext[kk * 128:(kk + 1) * 128, :])
                nc.vector.tensor_copy(wmb[:, kk, :], wf[:])
            for ms in range(ST):
                ps = p5po.tile([128, E], F32, tag="fc2", name=f"fc2_{ms}")
                for kk in range(FT):
                    for c in range(2):
                        ch = slice(c * NCH, (c + 1) * NCH)
                        nc.tensor.matmul(
                            ps[:, ch], h1[:, kk, ms * 128:(ms + 1) * 128],
                            wmb[:, kk, ch],
                            start=(kk == 0), stop=(kk == FT - 1))
                ot = p5o.tile([128, E], F32, tag="outsb", name=f"outsb{ms}")
                nc.vector.tensor_add(ot[:], ps[:], x2[:, ms, :])
                nc.sync.dma_start(out_ext[ms * 128:(ms + 1) * 128, :], ot[:])
            p5po.release()
            p5o.release()
            p5m.release()
            wmb_p.release()
            h1_p.release()
            yT_p.release()
            x2_p.release()
            cst.release()

    nc.finalize()
    return nc


_NC_CACHE = None


def kernel(**inputs) -> np.ndarray:
    global _NC_CACHE
    if _NC_CACHE is None:
        _NC_CACHE = build()
    nc = _NC_CACHE

    f32 = lambda a: np.ascontiguousarray(np.asarray(a), dtype=np.float32)
    base = {
        "ln1_scale": f32(inputs["ln1_scale"]),
        "wq": f32(inputs["wq"]).reshape(E, HD),
        "wk": f32(inputs["wk"]).reshape(E, HD),
        "wv": f32(inputs["wv"]).reshape(E, HD),
        "q_ln_scale": f32(inputs["q_ln_scale"]),
        "k_ln_scale": f32(inputs["k_ln_scale"]),
        "wo": f32(inputs["wo"]).reshape(HD, E),
        "ln2_scale": f32(inputs["ln2_scale"]),
        "wi": f32(inputs["wi"]),
        "wo_mlp": f32(inputs["wo_mlp"]),
    }
    lat = f32(inputs["latents"])
    in_maps = [dict(base, latents=np.ascontiguousarray(lat[i])) for i in range(B)]
    res = run_bass_kernel_spmd(nc, in_maps, list(range(B)))
    return np.stack([res.results[i]["out"] for i in range(B)], axis=0)
